# revision 21
# baseline (speedup 1.0000x reference)
"""Trainium2 Bass kernel for nn_CrossAttention (B=8, QL=KVL=2048, E=1024).

Sharding: data-parallel over batch — batch b runs on NeuronCore b.

Per-core dataflow, all-bf16 operands (fp32 PSUM accumulation):
  P1: qT, kT -> SBUF-resident bf16 [feat-part, seq]; v -> SBUF-resident
      bf16 [seq-part, feat], K and V computed in one pass over xkv.
  P2, per 512-wide q block: sT = kT.T@qT (PSUM f32), pT = exp(sT/32)
      (bf16) * mask, row sums accumulated on DVE then one tiny matmul per
      128-q chunk, oT = v.T@pT, y = (oT.T@WoT)*recip + bo.
  No DRAM bounce: everything stays resident; the only phase-2 DMA is the
  (prefetched) mask stream in and y out.  Two HWDGE queues (sync + ACT)
  split weight/x loads so the PE primes in ~3us.
"""

import os
import sys

import numpy as np

for _p in ("/opt/trn_rl_repo", "/opt/pypackages"):
    if _p not in sys.path and os.path.isdir(_p):
        sys.path.append(_p)

import concourse.bass as bass
import concourse.mybir as mybir
import concourse.tile as tile
from concourse.bass_utils import run_bass_kernel_spmd
from concourse.vector_clock import ScopedClock

F32 = mybir.dt.float32
F32R = mybir.dt.float32r
BF16 = mybir.dt.bfloat16
AF = mybir.ActivationFunctionType
ALU = mybir.AluOpType


def _ensure_ntff_hook():
    """The agent image's antenv lacks axon_hooks, so the boot-time NTFF
    profile hook registration silently degraded. Recreate the module and
    register the ctypes-based hook against libaxon_pjrt.so so trace=True
    runs produce per-core NTFF profiles (HW exec time)."""
    try:
        from antenv.axon_hooks import get_axon_ntff_profile_hook  # noqa: F401

        return
    except ImportError:
        pass
    import contextlib
    import ctypes
    import types

    import antenv

    mod = types.ModuleType("antenv.axon_hooks")
    mod._hook = None

    def set_axon_ntff_profile_hook(h):
        mod._hook = h

    def get_axon_ntff_profile_hook():
        return mod._hook

    mod.set_axon_ntff_profile_hook = set_axon_ntff_profile_hook
    mod.get_axon_ntff_profile_hook = get_axon_ntff_profile_hook
    sys.modules["antenv.axon_hooks"] = mod
    antenv.axon_hooks = mod

    so_path = "/opt/axon/libaxon_pjrt.so"
    if not os.path.exists(so_path):
        return
    lib = ctypes.CDLL(so_path)
    if not hasattr(lib, "axon_start_nrt_profile"):
        return
    lib.axon_start_nrt_profile.argtypes = [
        ctypes.POINTER(ctypes.c_int64),
        ctypes.c_size_t,
    ]
    lib.axon_start_nrt_profile.restype = ctypes.c_int64
    lib.axon_stop_nrt_profile.argtypes = [ctypes.c_char_p]
    lib.axon_stop_nrt_profile.restype = ctypes.c_int64

    @contextlib.contextmanager
    def _hook(output_dir, device_ids):
        import jax

        jax.devices()
        if device_ids:
            ids = (ctypes.c_int64 * len(device_ids))(*device_ids)
            rc = lib.axon_start_nrt_profile(ids, len(device_ids))
        else:
            rc = lib.axon_start_nrt_profile(None, 0)
        if rc != 0:
            raise RuntimeError(f"axon_start_nrt_profile rc={rc}")
        try:
            yield
        finally:
            n = lib.axon_stop_nrt_profile(str(output_dir).encode())
            print(f"ntff profile: {n} file(s) written to {output_dir}")

    set_axon_ntff_profile_hook(_hook)


_ensure_ntff_hook()

B, QL, KVL, E = 8, 2048, 2048, 1024
P = 128
EC = E // P          # 8 feature chunks
SCALE = 1.0 / 32.0   # 1/sqrt(E)
QB = 512             # q block (moving-operand width) in phase 2
LB = 512             # x block in phase 1


class _TC(tile.TileContext):
    """TileContext whose final drain never carries >1 sync wait.

    The walrus build in this container rejects instructions with more than
    one sync-wait command; spread the drain's waits across single-wait NOPs.
    """

    def _drain_and_barrier(self, tick_clock, wait_clock):
        nc = self.nc
        probe = nc.sync.nop(nofuse=True, hint="drain_wait_probe")
        wait_clock.add_sem_waits(
            probe.ins, ScopedClock({None: tick_clock.global_clock})
        )
        si = probe.ins.sync_info
        waits = list(si.on_wait) if si is not None else []
        if len(waits) > 1:
            probe.ins.sync_info = mybir.SyncInfo(
                on_wait=waits[:1], on_update=list(si.on_update)
            )
            for w in waits[1:]:
                extra = nc.sync.nop(nofuse=True, hint="drain_wait_spill")
                extra.ins.sync_info = mybir.SyncInfo(on_wait=[w], on_update=[])
        nc.sync.drain()
        nc.all_engine_barrier()
        assert self.sems is not None
        popped = nc._tile_sem_poison_stack.pop()
        assert popped is self._sem_poison
        nc.clear_and_free_semaphores(list(self.sems.allocated().values()))
        nc.all_engine_barrier()


def _split_multi_waits(nc):
    """Walrus here allows only one sync-wait per instruction; hoist extras
    onto same-engine NOPs inserted immediately before."""
    idx = 0
    for fn in nc.m.functions:
        for blk in fn.blocks:
            out = []
            changed = False
            for inst in blk.instructions:
                si = inst.sync_info
                if si is not None and len(si.on_wait) > 1:
                    changed = True
                    waits = list(si.on_wait)
                    for w in waits[:-1]:
                        nop = mybir.InstNoOp(name=f"I-waitsplit-{idx}")
                        idx += 1
                        nop.engine = inst.engine
                        nop.sync_info = mybir.SyncInfo(on_wait=[w], on_update=[])
                        out.append(nop)
                    inst.sync_info = mybir.SyncInfo(
                        on_wait=[waits[-1]], on_update=list(si.on_update)
                    )
                out.append(inst)
            if changed:
                blk.instructions = out


class _WParts:
    """N [P, EC, w] part-tiles presented as one [P, EC, N*w] tensor.

    Each o-slice handed to the PE must stay inside one part.
    """

    def __init__(self, parts, width):
        self._p = parts
        self._w = width

    def __getitem__(self, key):
        p, ec, o = key
        if isinstance(o, slice):
            start, stop = o.start or 0, o.stop
            i = start // self._w
            assert stop <= (i + 1) * self._w
            return self._p[i][p, ec, start - i * self._w : stop - i * self._w]
        raise TypeError(o)


def _load_w_parts(nc, wpool, w_b, eng, nparts, tagpfx):
    """Load a host-blocked weight [P, nparts, EC, width] as nparts tiles.

    Host blocking makes each part contiguous per partition, so the DMA
    runs with large descriptors, and the first matmul only waits for the
    first part rather than the whole matrix.
    """
    width = 1024 // nparts
    parts = []
    for i in range(nparts):
        t = wpool.tile([P, EC, width], BF16, tag=f"{tagpfx}{i}")
        eng.dma_start(out=t[:], in_=w_b[:, i])
        parts.append(t)
    return _WParts(parts, width)


def build_nc(ql=QL, kvl=KVL):
    """Build the single-core Bass program (same program runs on all 8 cores)."""
    kc = kvl // P        # kv chunks of 128
    nqb = ql // QB       # q blocks in phase 2
    qq_n = QB // P       # 128-row subblocks per q block
    eo2_n = E // 512

    nc = bass.Bass("TRN2", target_bir_lowering=False, debug=False)

    # activations arrive host-blocked: [block, partition, e-chunk, block-col]
    xq = nc.dram_tensor(
        "xq_blk", [ql // LB, P, EC, LB], BF16, kind="ExternalInput"
    ).ap()
    xkv = nc.dram_tensor(
        "xkv_blk", [kvl // LB, P, EC, LB], BF16, kind="ExternalInput"
    ).ap()
    maskb = nc.dram_tensor(
        "maskblk", [nqb, P, kc, QB], BF16, kind="ExternalInput"
    ).ap()
    # weights arrive host-blocked [p, part, ec, width] so each part is a
    # contiguous per-partition DMA and the first matmul waits only for the
    # first 512KB part
    wq = nc.dram_tensor("wq_blk", [P, 8, EC, 128], BF16, kind="ExternalInput").ap()
    wk = nc.dram_tensor("wk_blk", [P, 8, EC, 128], BF16, kind="ExternalInput").ap()
    wv = nc.dram_tensor("wv_blk", [P, 2, EC, 512], BF16, kind="ExternalInput").ap()
    wo = nc.dram_tensor("wo_blk", [P, 2, EC, 512], BF16, kind="ExternalInput").ap()
    bq = nc.dram_tensor("bq_pp", [P, EC], F32, kind="ExternalInput").ap()
    bk = nc.dram_tensor("bk_pp", [P, EC], F32, kind="ExternalInput").ap()
    bvr = nc.dram_tensor("bv_rep", [P, E], F32, kind="ExternalInput").ap()
    bor = nc.dram_tensor("bo_rep", [P, E], F32, kind="ExternalInput").ap()
    ones_in = nc.dram_tensor("ones", [P, 4], F32R, kind="ExternalInput").ap()
    y = nc.dram_tensor("y", [ql, E], F32, kind="ExternalOutput").ap()

    with _TC(nc) as tc:
        with (
            tc.tile_pool(name="persist", bufs=1) as persist,
            tc.tile_pool(name="consts", bufs=1) as consts,
            tc.tile_pool(name="wvo", bufs=1) as wvo,
            tc.tile_pool(name="maskp", bufs=1) as maskp,
        ):
            kt = persist.tile([P, EC, kvl], BF16, tag="kt")
            qt = persist.tile([P, EC, ql], BF16, tag="qt")
            vv = persist.tile([P, kc, E], BF16, tag="vv")

            bq_sb = consts.tile([P, EC], F32, tag="bq")
            bk_sb = consts.tile([P, EC], F32, tag="bk")
            bvr_sb = consts.tile([P, E], F32, tag="bvr")
            bor_sb = consts.tile([P, E], F32, tag="bor")
            ones = consts.tile([P, 4], F32R, tag="ones")

            # ---------------- Phase 1: projections ----------------
            # sync queue: weights + biases (phase-1 critical path);
            # scalar(ACT) queue: x blocks.  The two HWDGE rings drain in
            # parallel so the first matmul starts after ~0.5MB + 1MB.
            with (
                tc.tile_pool(name="p1x", bufs=2) as xpool,
                tc.tile_pool(name="wqk", bufs=2) as wqk,
                tc.tile_pool(name="p1ps", bufs=3, space="PSUM") as pp1,
            ):
                # Startup priming: both HWDGE rings share the ~358GB/s HBM
                # port, so balance the first-matmul critical bytes across
                # them and trickle x in ec-pair chunks so the opening psum
                # group starts as soon as wq part 0 + the first pair land.
                xblk = xpool.tile([P, EC, LB], BF16, tag="x")
                for i in range(0, 6, 2):
                    nc.scalar.dma_start(
                        out=xblk[:, i : i + 2, :], in_=xq[0][:, i : i + 2, :]
                    )
                wq_parts = [
                    wqk.tile([P, EC, 128], BF16, tag=f"q{i}", name=f"wq_p{i}")
                    for i in range(8)
                ]
                for i in (0, 1):
                    nc.sync.dma_start(out=wq_parts[i][:], in_=wq[:, i])
                nc.sync.dma_start(out=xblk[:, 6:8, :], in_=xq[0][:, 6:8, :])
                for i in range(2, 8):
                    nc.sync.dma_start(out=wq_parts[i][:], in_=wq[:, i])
                wq_sb = _WParts(wq_parts, 128)
                nc.sync.dma_start(out=bq_sb[:], in_=bq)
                nc.sync.dma_start(out=bk_sb[:], in_=bk)

                # qT = (WqT.T @ xqT) + bq -> SBUF resident bf16
                for qlb in range(ql // LB):
                    if qlb > 0:
                        xblk = xpool.tile([P, EC, LB], BF16, tag="x")
                        nc.scalar.dma_start(out=xblk[:], in_=xq[qlb])
                    for eo in range(EC):
                        ps = pp1.tile([P, LB], F32, tag="ps")
                        for ei in range(EC):
                            nc.tensor.matmul(
                                ps[:],
                                lhsT=(wq_sb[:, ei, eo * P : (eo + 1) * P]),
                                rhs=(xblk[:, ei, :]),
                                start=(ei == 0),
                                stop=(ei == EC - 1),
                            )
                        nc.scalar.activation(
                            qt[:, eo, qlb * LB : (qlb + 1) * LB],
                            ps[:],
                            AF.Identity,
                            bias=bq_sb[:, eo : eo + 1],
                        )

                # kT and v in one pass over xkv.
                xblk = xpool.tile([P, EC, LB], BF16, tag="x")
                nc.scalar.dma_start(out=xblk[:], in_=xkv[0])
                wk_sb = _load_w_parts(nc, wqk, wk, nc.sync, 8, "q")
                wv_sb = _load_w_parts(nc, wvo, wv, nc.sync, 2, "vh")
                nc.sync.dma_start(out=bvr_sb[:], in_=bvr)
                nc.sync.dma_start(out=ones[:], in_=ones_in)
                # wo + bo + the first mask block stream in behind phase 1
                wo_sb = _load_w_parts(nc, wvo, wo, nc.sync, 2, "oh")
                nc.sync.dma_start(out=bor_sb[:], in_=bor)
                mt = {}
                mtile = maskp.tile([P, kvl // P, QB], BF16, tag="mask")
                nc.sync.dma_start(out=mtile[:], in_=maskb[0])
                mt[0] = mtile
                for kvb in range(kvl // LB):
                    if kvb > 0:
                        xblk = xpool.tile([P, EC, LB], BF16, tag="x")
                        nc.scalar.dma_start(out=xblk[:], in_=xkv[kvb])
                    # kT = (WkT.T @ xkvT) + bk -> SBUF resident bf16
                    for eo in range(EC):
                        ps = pp1.tile([P, LB], F32, tag="ps")
                        for ei in range(EC):
                            nc.tensor.matmul(
                                ps[:],
                                lhsT=(wk_sb[:, ei, eo * P : (eo + 1) * P]),
                                rhs=(xblk[:, ei, :]),
                                start=(ei == 0),
                                stop=(ei == EC - 1),
                            )
                        nc.scalar.activation(
                            kt[:, eo, kvb * LB : (kvb + 1) * LB],
                            ps[:],
                            AF.Identity,
                            bias=bk_sb[:, eo : eo + 1],
                        )
                    # v = (xkvT.T @ WvT) + bv -> SBUF resident, [kv, E] layout
                    for k2 in range(LB // P):
                        kvc = kvb * (LB // P) + k2
                        for eo2 in range(eo2_n):
                            ps = pp1.tile([P, 512], F32, tag="ps")
                            for ei in range(EC):
                                nc.tensor.matmul(
                                    ps[:],
                                    lhsT=(xblk[:, ei, k2 * P : (k2 + 1) * P]),
                                    rhs=(
                                        wv_sb[:, ei, eo2 * 512 : (eo2 + 1) * 512]
                                    ),
                                    start=(ei == 0),
                                    stop=(ei == EC - 1),
                                )
                            nc.vector.tensor_tensor(
                                vv[:, kvc, eo2 * 512 : (eo2 + 1) * 512],
                                ps[:],
                                bvr_sb[:, eo2 * 512 : (eo2 + 1) * 512],
                                ALU.add,
                            )

            # ---------------- Phase 2: attention + output ----------------
            with (
                tc.tile_pool(name="p2pt", bufs=1) as ptp,
                tc.tile_pool(name="p2o", bufs=1) as osp,
                tc.tile_pool(name="p2acc", bufs=1) as accp,
                tc.tile_pool(name="p2small", bufs=2) as smallp,
                tc.tile_pool(name="p2out", bufs=2) as outp,
                tc.tile_pool(name="p2ps_s", bufs=3, space="PSUM") as pss,
                tc.tile_pool(name="p2ps_rs", bufs=1, space="PSUM") as psr,
                tc.tile_pool(name="p2ps_o", bufs=2, space="PSUM") as pso,
                tc.tile_pool(name="p2ps_f", bufs=2, space="PSUM") as psf,
            ):
                for iqb in range(nqb):
                    pt = ptp.tile([P, kc, QB], BF16, tag="pt")
                    acc = accp.tile([P, QB], F32R, tag="acc")
                    for c in range(kc):
                        ps = pss.tile([P, QB], F32, tag="s")
                        for e in range(EC):
                            nc.tensor.matmul(
                                ps[:],
                                lhsT=(kt[:, e, c * P : (c + 1) * P]),
                                rhs=(qt[:, e, iqb * QB : (iqb + 1) * QB]),
                                start=(e == 0),
                                stop=(e == EC - 1),
                            )
                        nc.scalar.activation(
                            pt[:, c, :], ps[:], AF.Exp, scale=SCALE
                        )
                        nc.vector.tensor_tensor(
                            pt[:, c, :], pt[:, c, :], mt[iqb][:, c, :], ALU.mult
                        )
                        # running kv-chunk sum for the softmax denominator
                        if c == 0:
                            nc.vector.tensor_scalar_add(
                                acc[:], pt[:, 0, :], 0.0
                            )
                        else:
                            nc.vector.tensor_tensor(
                                acc[:], acc[:], pt[:, c, :], ALU.add
                            )
                    # prefetch the next mask block (slot freed by the
                    # multiplies above; needed only after this block's
                    # AV + output projection)
                    if iqb + 1 < nqb:
                        mtile = maskp.tile([P, kvl // P, QB], BF16, tag="mask")
                        nc.scalar.dma_start(out=mtile[:], in_=maskb[iqb + 1])
                        mt[iqb + 1] = mtile

                    osb = osp.tile([P, EC, QB], BF16, tag="o")

                    def av_group(m):
                        po = pso.tile([P, QB], F32, tag="o")
                        for c in range(kc):
                            nc.tensor.matmul(
                                po[:],
                                lhsT=(vv[:, c, m * P : (m + 1) * P]),
                                rhs=(pt[:, c, :]),
                                start=(c == 0),
                                stop=(c == kc - 1),
                            )
                        nc.scalar.activation(osb[:, m, :], po[:], AF.Copy)

                    av_group(0)
                    # row sums: one tiny matmul per 128-q chunk against the
                    # DVE-accumulated acc (cheap LDWEIGHTS vs 64 pt-chunks)
                    recip = smallp.tile([P, qq_n], F32, tag="recip")
                    for qq in range(qq_n):
                        rs = psr.tile([P, 4], F32, tag="rs")
                        nc.tensor.matmul(
                            rs[:],
                            lhsT=(acc[:, qq * P : (qq + 1) * P]),
                            rhs=(ones[:]),
                            start=True,
                            stop=True,
                        )
                        nc.vector.reciprocal(recip[:, qq : qq + 1], rs[:, 0:1])
                    for m in range(1, EC):
                        av_group(m)

                    for eo2 in range(eo2_n):
                        for qq in range(qq_n):
                            pf = psf.tile([P, 512], F32, tag="f")
                            for m in range(EC):
                                nc.tensor.matmul(
                                    pf[:],
                                    lhsT=(osb[:, m, qq * P : (qq + 1) * P]),
                                    rhs=(
                                        wo_sb[:, m, eo2 * 512 : (eo2 + 1) * 512]
                                    ),
                                    start=(m == 0),
                                    stop=(m == EC - 1),
                                )
                            ot = outp.tile([P, 512], F32, tag="out")
                            last = (
                                iqb == nqb - 1
                                and eo2 == eo2_n - 1
                                and qq == qq_n - 1
                            )
                            # final group: halve the post-matmul DVE+DMA
                            # chain so the kernel tail drains sooner
                            for h in range(2) if last else (0,):
                                w = 256 if last else 512
                                nc.vector.scalar_tensor_tensor(
                                    ot[:, h * w : h * w + w],
                                    pf[:, h * w : h * w + w],
                                    recip[:, qq : qq + 1],
                                    bor_sb[
                                        :,
                                        eo2 * 512 + h * w : eo2 * 512 + h * w + w,
                                    ],
                                    ALU.mult,
                                    ALU.add,
                                )
                                nc.sync.dma_start(
                                    out=y[
                                        iqb * QB
                                        + qq * P : iqb * QB
                                        + (qq + 1) * P,
                                        eo2 * 512
                                        + h * w : eo2 * 512
                                        + h * w
                                        + w,
                                    ],
                                    in_=ot[:, h * w : h * w + w],
                                )

    _split_multi_waits(nc)
    return nc


_NC_CACHE = {}


def _get_nc(ql=QL, kvl=KVL):
    key = (ql, kvl)
    if key not in _NC_CACHE:
        _NC_CACHE[key] = build_nc(ql=ql, kvl=kvl)
    return _NC_CACHE[key]


def _bf16(a):
    import ml_dtypes

    return np.ascontiguousarray(a).astype(ml_dtypes.bfloat16)


def _host_prep(query, key_value, attention_mask, Wq, bq, Wk, bk, Wv, bv, Wo, bo):
    """Build the 8 per-core input maps (numpy only)."""
    b, ql, e = query.shape
    kvl = key_value.shape[1]
    kc, nqb = kvl // P, ql // QB

    f32 = np.float32

    def wblk(W, nparts):
        # [P, nparts, EC, width]: part-contiguous per partition
        width = E // nparts
        return _bf16(W.T.reshape(EC, P, nparts, width).transpose(1, 2, 0, 3))

    shared = {
        "wq_blk": wblk(Wq, 8),
        "wk_blk": wblk(Wk, 8),
        "wv_blk": wblk(Wv, 2),
        "wo_blk": wblk(Wo, 2),
        "bq_pp": np.ascontiguousarray(bq.reshape(EC, P).T, dtype=f32),
        "bk_pp": np.ascontiguousarray(bk.reshape(EC, P).T, dtype=f32),
        "bv_rep": np.ascontiguousarray(np.broadcast_to(bv, (P, e)), dtype=f32),
        "bo_rep": np.ascontiguousarray(np.broadcast_to(bo, (P, e)), dtype=f32),
        "ones": np.ones((P, 4), dtype=f32),
    }
    in_maps = []
    for i in range(b):
        m = attention_mask[i].T.astype(f32)  # [kv, q]
        mblk = _bf16(m.reshape(kc, P, nqb, QB).transpose(2, 1, 0, 3))
        xqb = _bf16(
            query[i].T.reshape(EC, P, ql // LB, LB).transpose(2, 1, 0, 3)
        )
        xkvb = _bf16(
            key_value[i].T.reshape(EC, P, kvl // LB, LB).transpose(2, 1, 0, 3)
        )
        in_maps.append(
            dict(shared, xq_blk=xqb, xkv_blk=xkvb, maskblk=mblk)
        )
    return in_maps


def run(inputs, trace=False):
    """Run on 8 cores; returns (output [B, QL, E], BassKernelResults)."""
    nc = _get_nc()
    in_maps = _host_prep(**inputs)
    res = run_bass_kernel_spmd(
        nc, in_maps, list(range(8)), trace=trace, trace_cores=[0]
    )
    out = np.stack([res.results[i]["y"] for i in range(8)], axis=0)
    return out, res


def kernel(**inputs):
    out, _ = run(inputs, trace=False)
    return out


# revision 22
# speedup vs baseline: 1.0081x; 1.0081x over previous
"""Trainium2 Bass kernel for nn_CrossAttention (B=8, QL=KVL=2048, E=1024).

Sharding: data-parallel over batch — batch b runs on NeuronCore b.

Per-core dataflow, all-bf16 operands (fp32 PSUM accumulation):
  P1: qT, kT -> SBUF-resident bf16 [feat-part, seq]; v -> SBUF-resident
      bf16 [seq-part, feat], K and V computed in one pass over xkv.
  P2, per 512-wide q block: sT = kT.T@qT (PSUM f32), pT = exp(sT/32)
      (bf16) * mask, row sums accumulated on DVE then one tiny matmul per
      128-q chunk, oT = v.T@pT, y = (oT.T@WoT)*recip + bo.
  No DRAM bounce: everything stays resident; the only phase-2 DMA is the
  (prefetched) mask stream in and y out.  Two HWDGE queues (sync + ACT)
  split weight/x loads so the PE primes in ~3us.
"""

import os
import sys

import numpy as np

for _p in ("/opt/trn_rl_repo", "/opt/pypackages"):
    if _p not in sys.path and os.path.isdir(_p):
        sys.path.append(_p)

import concourse.bass as bass
import concourse.mybir as mybir
import concourse.tile as tile
from concourse.bass_utils import run_bass_kernel_spmd
from concourse.vector_clock import ScopedClock

F32 = mybir.dt.float32
F32R = mybir.dt.float32r
BF16 = mybir.dt.bfloat16
AF = mybir.ActivationFunctionType
ALU = mybir.AluOpType


def _ensure_ntff_hook():
    """The agent image's antenv lacks axon_hooks, so the boot-time NTFF
    profile hook registration silently degraded. Recreate the module and
    register the ctypes-based hook against libaxon_pjrt.so so trace=True
    runs produce per-core NTFF profiles (HW exec time)."""
    try:
        from antenv.axon_hooks import get_axon_ntff_profile_hook  # noqa: F401

        return
    except ImportError:
        pass
    import contextlib
    import ctypes
    import types

    import antenv

    mod = types.ModuleType("antenv.axon_hooks")
    mod._hook = None

    def set_axon_ntff_profile_hook(h):
        mod._hook = h

    def get_axon_ntff_profile_hook():
        return mod._hook

    mod.set_axon_ntff_profile_hook = set_axon_ntff_profile_hook
    mod.get_axon_ntff_profile_hook = get_axon_ntff_profile_hook
    sys.modules["antenv.axon_hooks"] = mod
    antenv.axon_hooks = mod

    so_path = "/opt/axon/libaxon_pjrt.so"
    if not os.path.exists(so_path):
        return
    lib = ctypes.CDLL(so_path)
    if not hasattr(lib, "axon_start_nrt_profile"):
        return
    lib.axon_start_nrt_profile.argtypes = [
        ctypes.POINTER(ctypes.c_int64),
        ctypes.c_size_t,
    ]
    lib.axon_start_nrt_profile.restype = ctypes.c_int64
    lib.axon_stop_nrt_profile.argtypes = [ctypes.c_char_p]
    lib.axon_stop_nrt_profile.restype = ctypes.c_int64

    @contextlib.contextmanager
    def _hook(output_dir, device_ids):
        import jax

        jax.devices()
        if device_ids:
            ids = (ctypes.c_int64 * len(device_ids))(*device_ids)
            rc = lib.axon_start_nrt_profile(ids, len(device_ids))
        else:
            rc = lib.axon_start_nrt_profile(None, 0)
        if rc != 0:
            raise RuntimeError(f"axon_start_nrt_profile rc={rc}")
        try:
            yield
        finally:
            n = lib.axon_stop_nrt_profile(str(output_dir).encode())
            print(f"ntff profile: {n} file(s) written to {output_dir}")

    set_axon_ntff_profile_hook(_hook)


_ensure_ntff_hook()

B, QL, KVL, E = 8, 2048, 2048, 1024
P = 128
EC = E // P          # 8 feature chunks
SCALE = 1.0 / 32.0   # 1/sqrt(E)
QB = 512             # q block (moving-operand width) in phase 2
LB = 512             # x block in phase 1


class _TC(tile.TileContext):
    """TileContext whose final drain never carries >1 sync wait.

    The walrus build in this container rejects instructions with more than
    one sync-wait command; spread the drain's waits across single-wait NOPs.
    """

    def _drain_and_barrier(self, tick_clock, wait_clock):
        nc = self.nc
        probe = nc.sync.nop(nofuse=True, hint="drain_wait_probe")
        wait_clock.add_sem_waits(
            probe.ins, ScopedClock({None: tick_clock.global_clock})
        )
        si = probe.ins.sync_info
        waits = list(si.on_wait) if si is not None else []
        if len(waits) > 1:
            probe.ins.sync_info = mybir.SyncInfo(
                on_wait=waits[:1], on_update=list(si.on_update)
            )
            for w in waits[1:]:
                extra = nc.sync.nop(nofuse=True, hint="drain_wait_spill")
                extra.ins.sync_info = mybir.SyncInfo(on_wait=[w], on_update=[])
        nc.sync.drain()
        nc.all_engine_barrier()
        assert self.sems is not None
        popped = nc._tile_sem_poison_stack.pop()
        assert popped is self._sem_poison
        nc.clear_and_free_semaphores(list(self.sems.allocated().values()))
        nc.all_engine_barrier()


def _split_multi_waits(nc):
    """Walrus here allows only one sync-wait per instruction; hoist extras
    onto same-engine NOPs inserted immediately before."""
    idx = 0
    for fn in nc.m.functions:
        for blk in fn.blocks:
            out = []
            changed = False
            for inst in blk.instructions:
                si = inst.sync_info
                if si is not None and len(si.on_wait) > 1:
                    changed = True
                    waits = list(si.on_wait)
                    for w in waits[:-1]:
                        nop = mybir.InstNoOp(name=f"I-waitsplit-{idx}")
                        idx += 1
                        nop.engine = inst.engine
                        nop.sync_info = mybir.SyncInfo(on_wait=[w], on_update=[])
                        out.append(nop)
                    inst.sync_info = mybir.SyncInfo(
                        on_wait=[waits[-1]], on_update=list(si.on_update)
                    )
                out.append(inst)
            if changed:
                blk.instructions = out


class _WParts:
    """N [P, EC, w] part-tiles presented as one [P, EC, N*w] tensor.

    Each o-slice handed to the PE must stay inside one part.
    """

    def __init__(self, parts, width):
        self._p = parts
        self._w = width

    def __getitem__(self, key):
        p, ec, o = key
        if isinstance(o, slice):
            start, stop = o.start or 0, o.stop
            i = start // self._w
            assert stop <= (i + 1) * self._w
            return self._p[i][p, ec, start - i * self._w : stop - i * self._w]
        raise TypeError(o)


def _load_w_parts(nc, wpool, w_b, eng, nparts, tagpfx):
    """Load a host-blocked weight [P, nparts, EC, width] as nparts tiles.

    Host blocking makes each part contiguous per partition, so the DMA
    runs with large descriptors, and the first matmul only waits for the
    first part rather than the whole matrix.
    """
    width = 1024 // nparts
    parts = []
    for i in range(nparts):
        t = wpool.tile([P, EC, width], BF16, tag=f"{tagpfx}{i}")
        eng.dma_start(out=t[:], in_=w_b[:, i])
        parts.append(t)
    return _WParts(parts, width)


def build_nc(ql=QL, kvl=KVL):
    """Build the single-core Bass program (same program runs on all 8 cores)."""
    kc = kvl // P        # kv chunks of 128
    nqb = ql // QB       # q blocks in phase 2
    qq_n = QB // P       # 128-row subblocks per q block
    eo2_n = E // 512

    nc = bass.Bass("TRN2", target_bir_lowering=False, debug=False)

    # activations arrive host-blocked: [block, partition, e-chunk, block-col]
    xq = nc.dram_tensor(
        "xq_blk", [ql // LB, P, EC, LB], BF16, kind="ExternalInput"
    ).ap()
    xkv = nc.dram_tensor(
        "xkv_blk", [kvl // LB, P, EC, LB], BF16, kind="ExternalInput"
    ).ap()
    maskb = nc.dram_tensor(
        "maskblk", [nqb, P, kc, QB], BF16, kind="ExternalInput"
    ).ap()
    # weights arrive host-blocked [p, part, ec, width] so each part is a
    # contiguous per-partition DMA and the first matmul waits only for the
    # first 512KB part
    wq = nc.dram_tensor("wq_blk", [P, 8, EC, 128], BF16, kind="ExternalInput").ap()
    wk = nc.dram_tensor("wk_blk", [P, 8, EC, 128], BF16, kind="ExternalInput").ap()
    wv = nc.dram_tensor("wv_blk", [P, 2, EC, 512], BF16, kind="ExternalInput").ap()
    wo = nc.dram_tensor("wo_blk", [P, 2, EC, 512], BF16, kind="ExternalInput").ap()
    bq = nc.dram_tensor("bq_pp", [P, EC], F32, kind="ExternalInput").ap()
    bk = nc.dram_tensor("bk_pp", [P, EC], F32, kind="ExternalInput").ap()
    bvr = nc.dram_tensor("bv_rep", [P, E], F32, kind="ExternalInput").ap()
    bor = nc.dram_tensor("bo_rep", [P, E], F32, kind="ExternalInput").ap()
    ones_in = nc.dram_tensor("ones", [P, 4], F32R, kind="ExternalInput").ap()
    y = nc.dram_tensor("y", [ql, E], F32, kind="ExternalOutput").ap()

    with _TC(nc) as tc:
        with (
            tc.tile_pool(name="persist", bufs=1) as persist,
            tc.tile_pool(name="consts", bufs=1) as consts,
            tc.tile_pool(name="wvo", bufs=1) as wvo,
            tc.tile_pool(name="maskp", bufs=1) as maskp,
        ):
            kt = persist.tile([P, EC, kvl], BF16, tag="kt")
            qt = persist.tile([P, EC, ql], BF16, tag="qt")
            vv = persist.tile([P, kc, E], BF16, tag="vv")

            bq_sb = consts.tile([P, EC], F32, tag="bq")
            bk_sb = consts.tile([P, EC], F32, tag="bk")
            bvr_sb = consts.tile([P, E], F32, tag="bvr")
            bor_sb = consts.tile([P, E], F32, tag="bor")
            ones = consts.tile([P, 4], F32R, tag="ones")

            # ---------------- Phase 1: projections ----------------
            # sync queue: weights + biases (phase-1 critical path);
            # scalar(ACT) queue: x blocks.  The two HWDGE rings drain in
            # parallel so the first matmul starts after ~0.5MB + 1MB.
            with (
                tc.tile_pool(name="p1x", bufs=2) as xpool,
                tc.tile_pool(name="wqk", bufs=2) as wqk,
                tc.tile_pool(name="p1ps", bufs=3, space="PSUM") as pp1,
            ):
                # Startup priming: both HWDGE rings share the ~358GB/s HBM
                # port, so balance the first-matmul critical bytes across
                # them and trickle x in ec-pair chunks so the opening psum
                # group starts as soon as wq part 0 + the first pair land.
                xblk = xpool.tile([P, EC, LB], BF16, tag="x")
                for i in range(0, 6, 2):
                    nc.scalar.dma_start(
                        out=xblk[:, i : i + 2, :], in_=xq[0][:, i : i + 2, :]
                    )
                wq_parts = [
                    wqk.tile([P, EC, 128], BF16, tag=f"q{i}", name=f"wq_p{i}")
                    for i in range(8)
                ]
                for i in (0, 1):
                    nc.sync.dma_start(out=wq_parts[i][:], in_=wq[:, i])
                nc.sync.dma_start(out=xblk[:, 6:8, :], in_=xq[0][:, 6:8, :])
                for i in range(2, 8):
                    nc.sync.dma_start(out=wq_parts[i][:], in_=wq[:, i])
                wq_sb = _WParts(wq_parts, 128)
                nc.sync.dma_start(out=bq_sb[:], in_=bq)
                nc.sync.dma_start(out=bk_sb[:], in_=bk)

                # qT = (WqT.T @ xqT) + bq -> SBUF resident bf16
                for qlb in range(ql // LB):
                    if qlb > 0:
                        xblk = xpool.tile([P, EC, LB], BF16, tag="x")
                        nc.scalar.dma_start(out=xblk[:], in_=xq[qlb])
                    for eo in range(EC):
                        ps = pp1.tile([P, LB], F32, tag="ps")
                        for ei in range(EC):
                            nc.tensor.matmul(
                                ps[:],
                                lhsT=(wq_sb[:, ei, eo * P : (eo + 1) * P]),
                                rhs=(xblk[:, ei, :]),
                                start=(ei == 0),
                                stop=(ei == EC - 1),
                            )
                        nc.scalar.activation(
                            qt[:, eo, qlb * LB : (qlb + 1) * LB],
                            ps[:],
                            AF.Identity,
                            bias=bq_sb[:, eo : eo + 1],
                        )

                # kT and v in one pass over xkv.
                xblk = xpool.tile([P, EC, LB], BF16, tag="x")
                nc.scalar.dma_start(out=xblk[:], in_=xkv[0])
                wk_sb = _load_w_parts(nc, wqk, wk, nc.sync, 8, "q")
                wv_sb = _load_w_parts(nc, wvo, wv, nc.sync, 2, "vh")
                nc.sync.dma_start(out=bvr_sb[:], in_=bvr)
                nc.sync.dma_start(out=ones[:], in_=ones_in)
                # wo + bo + the first mask block stream in behind phase 1
                wo_sb = _load_w_parts(nc, wvo, wo, nc.sync, 2, "oh")
                nc.sync.dma_start(out=bor_sb[:], in_=bor)
                mt = {}
                mtile = maskp.tile([P, kvl // P, QB], BF16, tag="mask")
                nc.sync.dma_start(out=mtile[:], in_=maskb[0])
                mt[0] = mtile
                for kvb in range(kvl // LB):
                    if kvb > 0:
                        xblk = xpool.tile([P, EC, LB], BF16, tag="x")
                        nc.scalar.dma_start(out=xblk[:], in_=xkv[kvb])
                    # kT = (WkT.T @ xkvT) + bk -> SBUF resident bf16
                    for eo in range(EC):
                        ps = pp1.tile([P, LB], F32, tag="ps")
                        for ei in range(EC):
                            nc.tensor.matmul(
                                ps[:],
                                lhsT=(wk_sb[:, ei, eo * P : (eo + 1) * P]),
                                rhs=(xblk[:, ei, :]),
                                start=(ei == 0),
                                stop=(ei == EC - 1),
                            )
                        nc.scalar.activation(
                            kt[:, eo, kvb * LB : (kvb + 1) * LB],
                            ps[:],
                            AF.Identity,
                            bias=bk_sb[:, eo : eo + 1],
                        )
                    # v = (xkvT.T @ WvT) + bv -> SBUF resident, [kv, E] layout
                    for k2 in range(LB // P):
                        kvc = kvb * (LB // P) + k2
                        for eo2 in range(eo2_n):
                            ps = pp1.tile([P, 512], F32, tag="ps")
                            for ei in range(EC):
                                nc.tensor.matmul(
                                    ps[:],
                                    lhsT=(xblk[:, ei, k2 * P : (k2 + 1) * P]),
                                    rhs=(
                                        wv_sb[:, ei, eo2 * 512 : (eo2 + 1) * 512]
                                    ),
                                    start=(ei == 0),
                                    stop=(ei == EC - 1),
                                )
                            nc.vector.tensor_tensor(
                                vv[:, kvc, eo2 * 512 : (eo2 + 1) * 512],
                                ps[:],
                                bvr_sb[:, eo2 * 512 : (eo2 + 1) * 512],
                                ALU.add,
                            )

            # ---------------- Phase 2: attention + output ----------------
            with (
                tc.tile_pool(name="p2pt", bufs=1) as ptp,
                tc.tile_pool(name="p2o", bufs=1) as osp,
                tc.tile_pool(name="p2acc", bufs=1) as accp,
                tc.tile_pool(name="p2small", bufs=2) as smallp,
                tc.tile_pool(name="p2out", bufs=2) as outp,
                tc.tile_pool(name="p2ps_s", bufs=2, space="PSUM") as pss,
                tc.tile_pool(name="p2ps_rs", bufs=2, space="PSUM") as psr,
                tc.tile_pool(name="p2ps_o", bufs=2, space="PSUM") as pso,
                tc.tile_pool(name="p2ps_f", bufs=2, space="PSUM") as psf,
            ):
                for iqb in range(nqb):
                    pt = ptp.tile([P, kc, QB], BF16, tag="pt")
                    acc = accp.tile([P, QB], F32R, tag="acc")
                    for c in range(kc):
                        ps = pss.tile([P, QB], F32, tag="s")
                        for e in range(EC):
                            nc.tensor.matmul(
                                ps[:],
                                lhsT=(kt[:, e, c * P : (c + 1) * P]),
                                rhs=(qt[:, e, iqb * QB : (iqb + 1) * QB]),
                                start=(e == 0),
                                stop=(e == EC - 1),
                            )
                        nc.scalar.activation(
                            pt[:, c, :], ps[:], AF.Exp, scale=SCALE
                        )
                        nc.vector.tensor_tensor(
                            pt[:, c, :], pt[:, c, :], mt[iqb][:, c, :], ALU.mult
                        )
                        # running kv-chunk sum for the softmax denominator
                        if c == 0:
                            nc.vector.tensor_scalar_add(
                                acc[:], pt[:, 0, :], 0.0
                            )
                        else:
                            nc.vector.tensor_tensor(
                                acc[:], acc[:], pt[:, c, :], ALU.add
                            )
                    # prefetch the next mask block (slot freed by the
                    # multiplies above; needed only after this block's
                    # AV + output projection)
                    if iqb + 1 < nqb:
                        mtile = maskp.tile([P, kvl // P, QB], BF16, tag="mask")
                        nc.scalar.dma_start(out=mtile[:], in_=maskb[iqb + 1])
                        mt[iqb + 1] = mtile

                    osb = osp.tile([P, EC, QB], BF16, tag="o")

                    def av_group(m):
                        po = pso.tile([P, QB], F32, tag="o")
                        for c in range(kc):
                            nc.tensor.matmul(
                                po[:],
                                lhsT=(vv[:, c, m * P : (m + 1) * P]),
                                rhs=(pt[:, c, :]),
                                start=(c == 0),
                                stop=(c == kc - 1),
                            )
                        nc.scalar.activation(osb[:, m, :], po[:], AF.Copy)

                    av_group(0)
                    # row sums: one tiny matmul per 128-q chunk against the
                    # DVE-accumulated acc (cheap LDWEIGHTS vs 64 pt-chunks)
                    recip = smallp.tile([P, qq_n], F32, tag="recip")
                    for qq in range(qq_n):
                        rs = psr.tile([P, 4], F32, tag="rs")
                        nc.tensor.matmul(
                            rs[:],
                            lhsT=(acc[:, qq * P : (qq + 1) * P]),
                            rhs=(ones[:]),
                            start=True,
                            stop=True,
                        )
                        nc.vector.reciprocal(recip[:, qq : qq + 1], rs[:, 0:1])
                    for m in range(1, EC):
                        av_group(m)

                    for eo2 in range(eo2_n):
                        for qq in range(qq_n):
                            pf = psf.tile([P, 512], F32, tag="f")
                            for m in range(EC):
                                nc.tensor.matmul(
                                    pf[:],
                                    lhsT=(osb[:, m, qq * P : (qq + 1) * P]),
                                    rhs=(
                                        wo_sb[:, m, eo2 * 512 : (eo2 + 1) * 512]
                                    ),
                                    start=(m == 0),
                                    stop=(m == EC - 1),
                                )
                            ot = outp.tile([P, 512], F32, tag="out")
                            last = (
                                iqb == nqb - 1
                                and eo2 == eo2_n - 1
                                and qq == qq_n - 1
                            )
                            # final group: halve the post-matmul DVE+DMA
                            # chain so the kernel tail drains sooner
                            for h in range(2) if last else (0,):
                                w = 256 if last else 512
                                nc.vector.scalar_tensor_tensor(
                                    ot[:, h * w : h * w + w],
                                    pf[:, h * w : h * w + w],
                                    recip[:, qq : qq + 1],
                                    bor_sb[
                                        :,
                                        eo2 * 512 + h * w : eo2 * 512 + h * w + w,
                                    ],
                                    ALU.mult,
                                    ALU.add,
                                )
                                nc.sync.dma_start(
                                    out=y[
                                        iqb * QB
                                        + qq * P : iqb * QB
                                        + (qq + 1) * P,
                                        eo2 * 512
                                        + h * w : eo2 * 512
                                        + h * w
                                        + w,
                                    ],
                                    in_=ot[:, h * w : h * w + w],
                                )

    _split_multi_waits(nc)
    return nc


_NC_CACHE = {}


def _get_nc(ql=QL, kvl=KVL):
    key = (ql, kvl)
    if key not in _NC_CACHE:
        _NC_CACHE[key] = build_nc(ql=ql, kvl=kvl)
    return _NC_CACHE[key]


def _bf16(a):
    import ml_dtypes

    return np.ascontiguousarray(a).astype(ml_dtypes.bfloat16)


def _host_prep(query, key_value, attention_mask, Wq, bq, Wk, bk, Wv, bv, Wo, bo):
    """Build the 8 per-core input maps (numpy only)."""
    b, ql, e = query.shape
    kvl = key_value.shape[1]
    kc, nqb = kvl // P, ql // QB

    f32 = np.float32

    def wblk(W, nparts):
        # [P, nparts, EC, width]: part-contiguous per partition
        width = E // nparts
        return _bf16(W.T.reshape(EC, P, nparts, width).transpose(1, 2, 0, 3))

    shared = {
        "wq_blk": wblk(Wq, 8),
        "wk_blk": wblk(Wk, 8),
        "wv_blk": wblk(Wv, 2),
        "wo_blk": wblk(Wo, 2),
        "bq_pp": np.ascontiguousarray(bq.reshape(EC, P).T, dtype=f32),
        "bk_pp": np.ascontiguousarray(bk.reshape(EC, P).T, dtype=f32),
        "bv_rep": np.ascontiguousarray(np.broadcast_to(bv, (P, e)), dtype=f32),
        "bo_rep": np.ascontiguousarray(np.broadcast_to(bo, (P, e)), dtype=f32),
        "ones": np.ones((P, 4), dtype=f32),
    }
    in_maps = []
    for i in range(b):
        m = attention_mask[i].T.astype(f32)  # [kv, q]
        mblk = _bf16(m.reshape(kc, P, nqb, QB).transpose(2, 1, 0, 3))
        xqb = _bf16(
            query[i].T.reshape(EC, P, ql // LB, LB).transpose(2, 1, 0, 3)
        )
        xkvb = _bf16(
            key_value[i].T.reshape(EC, P, kvl // LB, LB).transpose(2, 1, 0, 3)
        )
        in_maps.append(
            dict(shared, xq_blk=xqb, xkv_blk=xkvb, maskblk=mblk)
        )
    return in_maps


def run(inputs, trace=False):
    """Run on 8 cores; returns (output [B, QL, E], BassKernelResults)."""
    nc = _get_nc()
    in_maps = _host_prep(**inputs)
    res = run_bass_kernel_spmd(
        nc, in_maps, list(range(8)), trace=trace, trace_cores=[0]
    )
    out = np.stack([res.results[i]["y"] for i in range(8)], axis=0)
    return out, res


def kernel(**inputs):
    out, _ = run(inputs, trace=False)
    return out
